# revision 1
# baseline (speedup 1.0000x reference)
"""ComplexBatchNorm2D (per-channel 2x2 covariance whitening + affine) on 8 trn2 cores.

Sharding: by channel (C=256 -> 32 channels per core). Per-channel statistics are
fully local to one core, so no collectives are needed. Each core processes its
32 channels in 8 groups of 4; a group is a [128, 4096] f32 tile pair with
partition p = (c_local*32 + b) and free = H*W. Data stays SBUF-resident between
the stats pass and the whitening apply, so HBM traffic is one read + one write.
"""

import sys

sys.path.insert(0, "/opt/trn_rl_repo")

import numpy as np

B, C, H, W = 32, 256, 64, 64
N_CORES = 8
C_PER_CORE = C // N_CORES  # 32
GROUPS = 8  # per core
C_PER_GROUP = C_PER_CORE // GROUPS  # 4
HW = H * W  # 4096
N = B * HW  # elements per channel
EPS = 1e-5

_CACHE = {}
LAST_RESULTS = None  # BassKernelResults from the most recent run (for test.py)
TRACE = False  # set True from test.py to collect an NTFF profile


def _build():
    import concourse.mybir as mybir
    import concourse.tile as tile
    from concourse.bacc import Bacc

    f32 = mybir.dt.float32
    Alu = mybir.AluOpType
    Act = mybir.ActivationFunctionType

    nc = Bacc()
    xr_d = nc.dram_tensor("xr", (B, C_PER_CORE, HW), f32, kind="ExternalInput")
    xi_d = nc.dram_tensor("xi", (B, C_PER_CORE, HW), f32, kind="ExternalInput")
    gc_d = nc.dram_tensor("gcols", (GROUPS, 128, 6), f32, kind="ExternalInput")
    out_d = nc.dram_tensor("out", (B, C_PER_CORE, 2 * HW), f32, kind="ExternalOutput")

    # Block-diagonal ones: bd[p, m] = 1 iff p//32 == m//32. One fp32 matmul with
    # this both reduces each channel's 32 b-partitions and broadcasts the result
    # back to all 128 partitions.
    bd = np.zeros((128, 128), np.float32)
    for blk in range(C_PER_GROUP):
        bd[blk * 32 : (blk + 1) * 32, blk * 32 : (blk + 1) * 32] = 1.0
    bd_d = nc.inline_tensor(bd, "bdiag")

    with tile.TileContext(nc) as tc:
        with (
            tc.tile_pool(name="io", bufs=3) as io_pool,
            tc.tile_pool(name="ob", bufs=2) as ob_pool,
            # bufs=8 = one slot per group: small tiles are never reused, so
            # no slot-release waits ever land on the ops that write them
            tc.tile_pool(name="small", bufs=8) as small_pool,
            tc.tile_pool(name="singles", bufs=1) as singles,
            tc.tile_pool(name="ps", bufs=8, space="PSUM") as ps_pool,
        ):
            bd_t = singles.tile([128, 128], f32)
            dma_bd = nc.sync.dma_start(out=bd_t, in_=bd_d[:, :])
            gc_t = singles.tile([128, GROUPS, 6], f32)
            dma_gc = nc.sync.dma_start(
                out=gc_t, in_=gc_d[:, :, :].rearrange("g p s -> p g s")
            )

            for g in range(GROUPS):
                h = {}
                cs = g * C_PER_GROUP
                xr = io_pool.tile([128, HW], f32, tag="xr")
                xi = io_pool.tile([128, HW], f32, tag="xi")
                # SBUF side must stay a flat [128, F] AP (a split partition
                # dim mis-lowers); the DRAM side carries the 3D reorder.
                # Loads and stats run per hw-half so stats start at half-load.
                HH = HW // 2
                for hh in range(2):
                    f0 = hh * HH
                    nc.sync.dma_start(
                        out=xr[:, f0 : f0 + HH],
                        in_=xr_d[:, cs : cs + C_PER_GROUP, f0 : f0 + HH]
                        .rearrange("b c f -> c b f"),
                    )
                    nc.sync.dma_start(
                        out=xi[:, f0 : f0 + HH],
                        in_=xi_d[:, cs : cs + C_PER_GROUP, f0 : f0 + HH]
                        .rearrange("b c f -> c b f"),
                    )

                ob = ob_pool.tile([128, 2 * HW], f32, tag="ob")
                ob3 = ob[:, :].rearrange("p (f two) -> p f two", two=2)
                # dump targets for value-discarded elementwise results
                scr_a = ob[:, 0:HW]
                scr_v = ob[:, HW : 2 * HW]

                # --- stats: raw sums per partition, per hw-half --------------
                # one stats tile per engine (each has a single writer engine);
                # cols hold per-half partials, combined by PSUM-accumulating
                # matmuls below
                st_a = small_pool.tile([128, 4], f32, tag="st_a")  # ACT
                st_v = small_pool.tile([128, 6], f32, tag="st_v")  # DVE
                for hh in range(2):
                    f0, ca, cv = hh * HH, 2 * hh, 3 * hh
                    xr_h, xi_h = xr[:, f0 : f0 + HH], xi[:, f0 : f0 + HH]
                    scr_ah, scr_vh = scr_a[:, f0 : f0 + HH], scr_v[:, f0 : f0 + HH]
                    nc.scalar.activation(
                        scr_ah, xr_h, Act.Square, accum_out=st_a[:, ca : ca + 1]
                    )
                    nc.scalar.activation(
                        scr_ah, xi_h, Act.Square, accum_out=st_a[:, ca + 1 : ca + 2]
                    )
                    # sum_ri: product and free-axis sum fused in one DVE op
                    nc.vector.scalar_tensor_tensor(
                        scr_vh, xr_h, 1.0, xi_h, Alu.mult, Alu.mult,
                        accum_out=st_v[:, cv + 2 : cv + 3],
                    )
                    # plain sums at 2x rate on DVE
                    nc.vector.tensor_scalar(
                        scr_vh, xr_h, 1.0, 0.0, Alu.mult, Alu.add,
                        accum_out=st_v[:, cv : cv + 1],
                    )
                    nc.vector.tensor_scalar(
                        scr_vh, xi_h, 1.0, 0.0, Alu.mult, Alu.add,
                        accum_out=st_v[:, cv + 1 : cv + 2],
                    )

                # --- aggregate over b and broadcast back (block-diag matmul) --
                # ps cols: 0 sum_r, 1 sum_i, 2 sum_ri, 3 sum_rr, 4 sum_ii;
                # the second matmul of each pair accumulates the other half
                ps = ps_pool.tile([128, 5], f32, tag="ps")
                nc.tensor.matmul(ps[:, 3:5], bd_t, st_a[:, 0:2],
                                 start=True, stop=False)
                nc.tensor.matmul(ps[:, 3:5], bd_t, st_a[:, 2:4],
                                 start=False, stop=True)
                nc.tensor.matmul(ps[:, 0:3], bd_t, st_v[:, 0:3],
                                 start=True, stop=False)
                nc.tensor.matmul(ps[:, 0:3], bd_t, st_v[:, 3:6],
                                 start=False, stop=True)

                # T columns: 0 m_r, 1 m_i, 2 e_ri, 3 e_rr, 4 e_ii, 5 a, 6 d,
                # 7 nb, 8 s0, 9 1/s0, 10 det/s0, 11 s, 12 ad, 13 nb2, 14 det,
                # 15 tr2s, 16 t0, 17 1/t0, 18 tr2s/t0, 19 t, 20 dn, 21 rdn,
                # 22 dps, 23 aps, 24:26 gb, 26:28 ga, 28:30 A00|A10,
                # 30:32 A01|A11, 32:34 t6, 34:36 t7, 36:38 bias_r|bias_i
                # T cols 0..4 = m_r, m_i, e_ri, e_rr, e_ii (ps order matches)
                T = small_pool.tile([128, 38], f32, tag="T")
                h["evac3"] = nc.scalar.activation(
                    T[:, 0:5], ps[:, 0:5], Act.Copy, scale=1.0 / N
                )

                gc = gc_t[:, g, :]
                stt = nc.vector.scalar_tensor_tensor
                tt = nc.vector.tensor_tensor
                ts = nc.vector.tensor_scalar

                # a, d = E[x^2] - m^2 + 2*EPS  (reference adds EPS to cov twice)
                stt(T[:, 5:7], T[:, 0:2], -1.0, T[:, 0:2], Alu.mult, Alu.mult)
                stt(T[:, 5:7], T[:, 5:7], 2.0 * EPS, T[:, 3:5], Alu.add, Alu.add)
                # nb = -b = m_r*m_i - E[ri]
                stt(T[:, 7:8], T[:, 0:1], T[:, 1:2], T[:, 2:3], Alu.mult, Alu.subtract)
                # det = a*d - b^2
                tt(T[:, 12:13], T[:, 5:6], T[:, 6:7], Alu.mult)
                tt(T[:, 13:14], T[:, 7:8], T[:, 7:8], Alu.mult)
                tt(T[:, 14:15], T[:, 12:13], T[:, 13:14], Alu.subtract)
                # s = sqrt(det), Newton-refined (ACT sqrt LUT is low-precision)
                nc.scalar.activation(T[:, 8:9], T[:, 14:15], Act.Sqrt)
                nc.vector.reciprocal(T[:, 9:10], T[:, 8:9])
                tt(T[:, 10:11], T[:, 14:15], T[:, 9:10], Alu.mult)
                tt(T[:, 11:12], T[:, 8:9], T[:, 10:11], Alu.add)
                ts(T[:, 11:12], T[:, 11:12], 0.5, None, Alu.mult)
                # dps = d+s, aps = a+s, tr2s = a+d+2s
                tt(T[:, 22:23], T[:, 6:7], T[:, 11:12], Alu.add)
                tt(T[:, 23:24], T[:, 5:6], T[:, 11:12], Alu.add)
                tt(T[:, 15:16], T[:, 22:23], T[:, 23:24], Alu.add)
                # t = sqrt(tr2s), Newton-refined
                nc.scalar.activation(T[:, 16:17], T[:, 15:16], Act.Sqrt)
                nc.vector.reciprocal(T[:, 17:18], T[:, 16:17])
                tt(T[:, 18:19], T[:, 15:16], T[:, 17:18], Alu.mult)
                tt(T[:, 19:20], T[:, 16:17], T[:, 18:19], Alu.add)
                ts(T[:, 19:20], T[:, 19:20], 0.5, None, Alu.mult)
                # rdn = 1/(s*t)
                tt(T[:, 20:21], T[:, 11:12], T[:, 19:20], Alu.mult)
                nc.vector.reciprocal(T[:, 21:22], T[:, 20:21])
                # A = gamma @ W, W = [[dps, nb], [nb, aps]] * rdn
                # [A00, A10] = ([g00,g10]*dps + [g01,g11]*nb) * rdn
                ts(T[:, 24:26], gc[:, 2:4], T[:, 7:8], None, Alu.mult)
                stt(T[:, 28:30], gc[:, 0:2], T[:, 22:23], T[:, 24:26], Alu.mult, Alu.add)
                ts(T[:, 28:30], T[:, 28:30], T[:, 21:22], None, Alu.mult)
                # [A01, A11] = ([g00,g10]*nb + [g01,g11]*aps) * rdn
                ts(T[:, 26:28], gc[:, 2:4], T[:, 23:24], None, Alu.mult)
                stt(T[:, 30:32], gc[:, 0:2], T[:, 7:8], T[:, 26:28], Alu.mult, Alu.add)
                ts(T[:, 30:32], T[:, 30:32], T[:, 21:22], None, Alu.mult)
                # bias' = beta - [A00,A10]*m_r - [A01,A11]*m_i
                ts(T[:, 32:34], T[:, 28:30], T[:, 0:1], None, Alu.mult)
                stt(T[:, 34:36], T[:, 30:32], T[:, 1:2], T[:, 32:34], Alu.mult, Alu.add)
                tt(T[:, 36:38], gc[:, 4:6], T[:, 34:36], Alu.subtract)

                # --- apply: out_r = A00*xr + A01*xi + br'; interleave r/i -----
                # u_r = A00*xr + br -> ob second half (read leads write in the
                # later strided STT, so the overlap is safe); u_i = A10*xr + bi
                # -> in place over xr (xr's last use). Keeps ACT decoupled
                # from the DVE ob writes with zero extra SBUF.
                u_r = scr_v
                nc.scalar.activation(
                    u_r, xr, Act.Identity, bias=T[:, 36:37], scale=T[:, 28:29]
                )
                h["a3"] = nc.scalar.activation(
                    xr, xr, Act.Identity, bias=T[:, 37:38], scale=T[:, 29:30]
                )
                # DVE apply runs per hw-half so the first half's 2 MiB store
                # can launch while the second half still computes
                HH = HW // 2
                for hh in range(2):
                    f0 = hh * HH
                    stt(
                        ob3[:, f0 : f0 + HH, 0], xi[:, f0 : f0 + HH],
                        T[:, 30:31], u_r[:, f0 : f0 + HH], Alu.mult, Alu.add,
                    )
                    stt(
                        ob3[:, f0 : f0 + HH, 1], xi[:, f0 : f0 + HH],
                        T[:, 31:32], xr[:, f0 : f0 + HH], Alu.mult, Alu.add,
                    )
                    nc.sync.dma_start(
                        out=out_d[:, cs : cs + C_PER_GROUP, 2 * f0 : 2 * f0 + HW]
                        .rearrange("b c f -> c b f"),
                        in_=ob[:, 2 * f0 : 2 * f0 + HW],
                    )
    nc.finalize()
    return nc


def kernel(x_real, x_imag, gamma, beta):
    global LAST_RESULTS
    from concourse.bass_utils import run_bass_kernel_spmd

    if "nc" not in _CACHE:
        _CACHE["nc"] = _build()
    nc = _CACHE["nc"]

    x_real = np.asarray(x_real, dtype=np.float32)
    x_imag = np.asarray(x_imag, dtype=np.float32)
    gamma = np.asarray(gamma, dtype=np.float32)
    beta = np.asarray(beta, dtype=np.float32)

    # per-channel columns [g00, g10, g01, g11, beta_r, beta_i]
    gcols_all = np.stack(
        [gamma[:, 0, 0], gamma[:, 1, 0], gamma[:, 0, 1], gamma[:, 1, 1],
         beta[:, 0], beta[:, 1]],
        axis=-1,
    ).astype(np.float32)  # (C, 6)

    in_maps = []
    for k in range(N_CORES):
        sl = slice(k * C_PER_CORE, (k + 1) * C_PER_CORE)
        gk = gcols_all[sl].reshape(GROUPS, C_PER_GROUP, 1, 6)
        gk = np.broadcast_to(gk, (GROUPS, C_PER_GROUP, 32, 6)).reshape(GROUPS, 128, 6)
        in_maps.append(
            {
                "xr": np.ascontiguousarray(x_real[:, sl].reshape(B, C_PER_CORE, HW)),
                "xi": np.ascontiguousarray(x_imag[:, sl].reshape(B, C_PER_CORE, HW)),
                "gcols": np.ascontiguousarray(gk),
            }
        )

    res = run_bass_kernel_spmd(
        nc, in_maps, core_ids=list(range(N_CORES)), trace=TRACE
    )
    LAST_RESULTS = res

    out = np.empty((B, C, H, W, 2), dtype=np.float32)
    for k in range(N_CORES):
        sl = slice(k * C_PER_CORE, (k + 1) * C_PER_CORE)
        out[:, sl] = res.results[k]["out"].reshape(B, C_PER_CORE, H, W, 2)
    return out

